# revision 1
# baseline (speedup 1.0000x reference)
"""NoisyHadamardLinear Trainium2 kernel (self-contained).

y = blockwise_FHT_1024(x) @ W^T + b  for x [2, 4096, 4096], W [4096, 4096],
b [4096], on 8 NeuronCores, data-parallel over the 8192 tokens (1024/core).

Per-core pipeline (all matmuls fp32r on TensorE):
  phase H: PE-transpose x tiles -> xT chunks; apply H_128/32 as one matmul
           per 128-chunk with butterfly stage-1 folded into the PSUM
           accumulation (H_1024 = H_8 (x) H_128 Kronecker); butterfly
           stages 2-3 on VectorE -> xhT tiles [d, t] resident in SBUF.
  phase M: per 512-wide o-slab, PE-transpose W tiles on the fly -> WT;
           y[t, o] = sum_d xhT[d, t].T @ WT[d, o] accumulated over 32
           d-tiles in PSUM + bias rank-1 (ones x b) matmul; ACT evict; DMA.
"""
import numpy as np

import concourse.bacc as bacc
import concourse.mybir as mybir
import concourse.tile as tile
from concourse.bass_utils import run_bass_kernel_spmd

P = 128
f32r = mybir.dt.float32r
f32 = mybir.dt.float32

N_CORES = 8
B, S, D, O = 2, 4096, 4096, 4096
T_PER_CORE = (B * S) // N_CORES


def build_kernel(T=T_PER_CORE, D=D, O=O, OS=512, num_devices=N_CORES,
                 phases=('H', 'M')):
    NTH = 2 if T >= 1024 else 1            # t-halves
    TH = T // NTH                          # tokens per half
    NTS = TH // P                          # t-subtiles per half
    NBLK = D // 1024                       # hadamard blocks
    ND = D // P                            # d tiles
    NOS = O // OS                          # o-slabs
    NOSUB = OS // P                        # o-subtiles per slab

    nc = bacc.Bacc("TRN2", target_bir_lowering=False, debug=False,
                   num_devices=num_devices, dynamic_dma_scratch_size=2048)
    x = nc.dram_tensor("x", [T, D], f32r, kind="ExternalInput")
    W = nc.dram_tensor("W", [O, D], f32r, kind="ExternalInput")
    b = nc.dram_tensor("b", [1, O], f32r, kind="ExternalInput")
    Hp = nc.dram_tensor("Hp", [P, P], f32r, kind="ExternalInput")
    Hn = nc.dram_tensor("Hn", [P, P], f32r, kind="ExternalInput")
    Ident = nc.dram_tensor("Ident", [P, P], f32r, kind="ExternalInput")
    Ones = nc.dram_tensor("Ones", [1, P], f32r, kind="ExternalInput")
    y = nc.dram_tensor("y", [T, O], f32, kind="ExternalOutput")

    with tile.TileContext(nc) as tc:
        with tc.tile_pool(name="const", bufs=1) as cpool, \
             tc.tile_pool(name="xhT", bufs=ND) as xhTp:
            ident = cpool.tile([P, P], f32r)
            hp = cpool.tile([P, P], f32r)
            hn = cpool.tile([P, P], f32r)
            ones = cpool.tile([1, P], f32r)
            nc.sync.dma_start(ident[:], Ident.ap())
            nc.sync.dma_start(hp[:], Hp.ap())
            nc.sync.dma_start(hn[:], Hn.ap())
            nc.sync.dma_start(ones[:], Ones.ap())

            # persistent xhT tiles [128 d, T tokens]
            xhT = [xhTp.tile([P, T], f32r, tag="xhT", name=f"xhT{i}")
                   for i in range(ND)]

            if 'H' in phases:
                _phase_h(nc, tc, x, ident, hp, hn, xhT,
                         NTH, TH, NTS, NBLK)
            if 'M' in phases:
                _phase_m(nc, tc, W, b, ident, ones, xhT, y,
                         NTH, NTS, ND, NOS, NOSUB, OS, D)
    nc.compile()
    return nc


def _phase_h(nc, tc, x, ident, hp, hn, xhT, NTH, TH, NTS, NBLK):
    with tc.tile_pool(name="xnat", bufs=NTS + 1) as xnat, \
         tc.tile_pool(name="xTp", bufs=9) as xTp, \
         tc.tile_pool(name="bfp", bufs=20) as bfp, \
         tc.tile_pool(name="tps", bufs=4, space="PSUM") as tps, \
         tc.tile_pool(name="hps", bufs=4, space="PSUM") as hps:
        for th in range(NTH):
            for blk in range(NBLK):
                xns = []
                for ts in range(NTS):
                    xn = xnat.tile([P, 1024], f32r, tag="xn")
                    trow = (th * NTS + ts) * P
                    nc.sync.dma_start(
                        xn[:], x.ap()[trow:trow + P,
                                      blk * 1024:(blk + 1) * 1024])
                    xns.append(xn)
                # transpose x tiles -> xT chunks
                xTs = []
                for u in range(8):
                    tp = tps.tile([P, TH], f32r, tag="tps")
                    for ts in range(NTS):
                        nc.tensor.transpose(
                            tp[:, ts * P:(ts + 1) * P],
                            xns[ts][:, u * P:(u + 1) * P], ident[:])
                    t = xTp.tile([P, TH], f32r, tag="xT")
                    nc.scalar.copy(t[:], tp[:])
                    xTs.append(t)
                # H128/32 chunk matmuls with butterfly stage-1 folded into
                # PSUM accumulation: s_k = H(x_2k)+H(x_2k+1),
                # d_k = H(x_2k)-H(x_2k+1) (via -H on the second operand)
                cur = []
                for k in range(4):
                    for sign in range(2):
                        ph = hps.tile([P, TH], f32, tag="hps")
                        nc.tensor.matmul(ph[:], hp[:], xTs[2 * k][:],
                                         start=True, stop=False)
                        nc.tensor.matmul(ph[:],
                                         (hp if sign == 0 else hn)[:],
                                         xTs[2 * k + 1][:],
                                         start=False, stop=True)
                        z = bfp.tile([P, TH], f32r, tag="bf",
                                     name=f"z{th}_{blk}_{k}_{sign}")
                        nc.scalar.copy(z[:], ph[:])
                        cur.append(z)
                # H8 butterfly stages 2-3 on VectorE
                for s in range(1, 3):
                    stride = 1 << s
                    nxt = [bfp.tile([P, TH], f32r, tag="bf",
                                    name=f"bf{th}_{blk}_{s}_{v}")
                           if s < 2 else None
                           for v in range(8)]
                    for g in range(0, 8, 2 * stride):
                        for j in range(stride):
                            a = cur[g + j]
                            bb = cur[g + j + stride]
                            if s == 2:
                                oa = xhT[blk * 8 + g + j][
                                    :, th * TH:(th + 1) * TH]
                                ob = xhT[blk * 8 + g + j + stride][
                                    :, th * TH:(th + 1) * TH]
                            else:
                                oa = nxt[g + j][:]
                                ob = nxt[g + j + stride][:]
                            nc.vector.tensor_add(oa, a[:], bb[:])
                            nc.vector.tensor_sub(ob, a[:], bb[:])
                    cur = nxt


def _phase_m(nc, tc, W, b, ident, ones, xhT, y,
             NTH, NTS, ND, NOS, NOSUB, OS, D):
    NWCH = D // 512
    with tc.tile_pool(name="wnat", bufs=NOSUB + 1) as wnat, \
         tc.tile_pool(name="WTp", bufs=ND + 2) as WTp, \
         tc.tile_pool(name="bpool", bufs=2) as bpool, \
         tc.tile_pool(name="yout", bufs=2) as yout, \
         tc.tile_pool(name="tps", bufs=5, space="PSUM") as tps, \
         tc.tile_pool(name="yps", bufs=3, space="PSUM") as yps:
        for os_ in range(NOS):
            bt = bpool.tile([1, OS], f32r, tag="bt")
            nc.sync.dma_start(bt[:], b.ap()[:, os_ * OS:(os_ + 1) * OS])
            WTs = []
            for dch in range(NWCH):
                wns = []
                for osub in range(NOSUB):
                    wn = wnat.tile([P, 512], f32r, tag="wn")
                    orow = os_ * OS + osub * P
                    nc.sync.dma_start(
                        wn[:], W.ap()[orow:orow + P,
                                      dch * 512:(dch + 1) * 512])
                    wns.append(wn)
                for dt in range(4):
                    tp = tps.tile([P, OS], f32r, tag="tps")
                    for osub in range(NOSUB):
                        nc.tensor.transpose(
                            tp[:, osub * P:(osub + 1) * P],
                            wns[osub][:, dt * P:(dt + 1) * P], ident[:])
                    t = WTp.tile([P, OS], f32r, tag="WT")
                    if (dch * 4 + dt) % 2 == 0:
                        nc.vector.tensor_copy(t[:], tp[:])
                    else:
                        nc.scalar.copy(t[:], tp[:])
                    WTs.append(t)
            for ts in range(NTH * NTS):
                py = yps.tile([P, OS], f32, tag="yps")
                nc.tensor.matmul(py[:], ones[:1, :], bt[:1, :],
                                 start=True, stop=False)
                for d in range(ND):
                    nc.tensor.matmul(py[:], xhT[d][:, ts * P:(ts + 1) * P],
                                     WTs[d][:],
                                     start=False, stop=(d == ND - 1))
                yo = yout.tile([P, OS], f32, tag="yo")
                nc.scalar.copy(yo[:], py[:])
                nc.sync.dma_start(
                    y.ap()[ts * P:(ts + 1) * P,
                           os_ * OS:(os_ + 1) * OS], yo[:])

_CACHED_NC = None


def _get_nc():
    global _CACHED_NC
    if _CACHED_NC is None:
        _CACHED_NC = build_kernel()
    return _CACHED_NC


def _hadamard128():
    h = np.array([[1.0]], dtype=np.float32)
    while h.shape[0] < P:
        h = np.block([[h, h], [h, -h]])
    return h.astype(np.float32)


def kernel(x, W, b):
    x = np.asarray(x, dtype=np.float32)
    W = np.asarray(W, dtype=np.float32)
    b = np.asarray(b, dtype=np.float32)
    assert x.shape == (B, S, D) and W.shape == (O, D) and b.shape == (O,)

    nc = _get_nc()
    h128 = _hadamard128()
    consts = {
        "Hp": (h128 / 32.0).astype(np.float32),
        "Hn": (-h128 / 32.0).astype(np.float32),
        "Ident": np.eye(P, dtype=np.float32),
        "Ones": np.ones((1, P), np.float32),
    }
    xf = np.ascontiguousarray(x.reshape(B * S, D))
    in_maps = []
    for c in range(N_CORES):
        in_maps.append({
            "x": np.ascontiguousarray(xf[c * T_PER_CORE:(c + 1) * T_PER_CORE]),
            "W": W,
            "b": np.ascontiguousarray(b.reshape(1, O)),
            **consts,
        })
    res = run_bass_kernel_spmd(nc, in_maps, core_ids=list(range(N_CORES)))
    y = np.concatenate([res.results[c]["y"] for c in range(N_CORES)], axis=0)
    return y.reshape(B, S, O).astype(np.float32, copy=False)



# revision 3
# speedup vs baseline: 1.1697x; 1.1697x over previous
"""NoisyHadamardLinear TRN2 kernel v3: fp8 DoubleRow 3-pass, stage-1-folded FHT.

Differences from v2: butterfly stage 1 folded into the H128 PE matmul
(+-H accumulation), so DVE only runs stages 2-3; R residual on GPSIMD for
token-half 1 (hidden under main matmul) and DVE for half 0; main-matmul
chains ordered A-passes first, R-passes last, to shorten the startup
dependency on R.
"""
import numpy as np
import ml_dtypes

import concourse.bacc as bacc
import concourse.mybir as mybir
import concourse.tile as tile
from concourse.bass_utils import run_bass_kernel_spmd

P = 128
bf16 = mybir.dt.bfloat16
f32 = mybir.dt.float32
f8 = mybir.dt.float8e4

N_CORES = 8
B, S, D, O = 2, 4096, 4096, 4096
T = (B * S) // N_CORES
NBLK = D // 1024
NKP = D // 256
NTH = 2
TH = T // NTH
NOS = O // 512
KSCALE = 1024.0

E4 = ml_dtypes.float8_e4m3
BF = ml_dtypes.bfloat16
ACT_F = mybir.ActivationFunctionType


def _phase_h(nc, th, xT, hp, hn, A, R, xinp, xhp, hps):
    for blk in range(NBLK):
        _phase_h_blk(nc, th, blk, xT, hp, hn, A, R, xinp, xhp, hps)


def _phase_h_blk(nc, th, blk, xT, hp, hn, A, R, xinp, xhp, hps,
                 t0=None, tw=None):
    if t0 is None:
        t0, tw = th * TH, TH
    ts = slice(t0, t0 + tw)
    if True:
        xin = xinp.tile([P, 8, tw], bf16, tag="xin", name=f"xin{th}_{blk}_{t0}")
        nc.sync.dma_start(xin[:], xT.ap()[blk, :, :, ts])
        # H128 matmul with stage-1 butterfly folded into PSUM accumulation:
        # ps[2k]   = H x_{2k} + H x_{2k+1}
        # ps[2k+1] = H x_{2k} - H x_{2k+1}
        ps = []
        for k in range(4):
            for sign in range(2):
                p_ = hps.tile([P, tw], f32, tag="hps",
                              name=f"hps{th}_{blk}_{t0}_{k}_{sign}")
                nc.tensor.matmul(p_[:], hp[:], xin[:, 2 * k, :],
                                 start=True, stop=False)
                nc.tensor.matmul(p_[:], (hp if sign == 0 else hn)[:],
                                 xin[:, 2 * k + 1, :], start=False, stop=True)
                ps.append(p_)
        # HW allows only one PSUM input per vector op: copy the second
        # operand of each stage-2 pair to SBUF first (alternate ACT/DVE).
        sc = {}
        for n, i in enumerate((2, 3, 6, 7)):
            sc[i] = xhp.tile([P, tw], bf16, tag="sc",
                             name=f"sc_{th}_{blk}_{t0}_{i}")
            nc.scalar.activation(sc[i][:], ps[i][:], ACT_F.Copy)
        # stage 2 (stride 2): PSUM + SBUF -> bf16 SBUF
        s2 = [xhp.tile([P, tw], bf16, tag="xh", name=f"s2_{th}_{blk}_{t0}_{i}")
              for i in range(8)]
        for g in (0, 4):
            for j in range(2):
                nc.vector.tensor_add(s2[g + j][:], ps[g + j][:],
                                     sc[g + j + 2][:])
                nc.vector.tensor_sub(s2[g + j + 2][:], ps[g + j][:],
                                     sc[g + j + 2][:])
        # stage 3 (stride 4) -> xh bf16
        s3 = [xhp.tile([P, tw], bf16, tag="xh", name=f"s3_{th}_{blk}_{t0}_{i}")
              for i in range(8)]
        for j in range(4):
            eng = nc.gpsimd if (th == 0 and j >= 2) else nc.vector
            eng.tensor_add(s3[j][:], s2[j][:], s2[j + 4][:])
            eng.tensor_sub(s3[j + 4][:], s2[j][:], s2[j + 4][:])
        # A = fp8(xh) on ACT; R = xh - A.  For the startup-critical first
        # half, split R between DVE and GPSIMD; for the second half (hidden
        # under the main matmul) run R entirely on GPSIMD.
        for c in range(8):
            ch = blk * 8 + c
            kp, sl = ch // 2, ch % 2
            a_ap = A[kp][:, sl, ts]
            nc.scalar.activation(a_ap, s3[c][:], ACT_F.Copy)
            if th == 0:
                eng = nc.vector if c % 2 == 0 else nc.gpsimd
            else:
                eng = nc.gpsimd
            eng.tensor_sub(R[kp][:, sl, ts], s3[c][:], a_ap)


def _phase_m(nc, th, W1s, W2s, btile, A, R, y, wp1, wp2, yop, mps):
    for osb in range(NOS):
        _phase_m_osb(nc, th, osb, W1s, W2s, btile, A, R, y,
                     wp1, wp2, yop, mps)


def _m_chain(nc, py, w1, w2, A, R, t0, tw, ot):
    """One PSUM group: A@W1 + A@W2 + R@W1 over tokens [t0, t0+tw)."""
    first = True
    ntc = tw // 256
    for xt, wt, isr in ((A, w1, 0), (A, w2, 0), (R, w1, 1)):
        for kp in range(NKP):
            for tc in range(ntc):
                out_ap = py[:, tc * 256:(tc + 1) * 256]
                tcs = slice(t0 + tc * 256, t0 + (tc + 1) * 256)
                last = (isr == 1 and kp == NKP - 1 and tc == ntc - 1)
                nc.tensor.matmul(
                    out_ap, wt[:, kp, :, ot * P:(ot + 1) * P],
                    xt[kp][:, :, tcs],
                    start=first, stop=last,
                    perf_mode=mybir.MatmulPerfMode.DoubleRow)
                first = False


def _m_load_w(nc, osb, W1s, W2s, wp1, wp2, tag):
    w1 = wp1.tile([P, NKP, 2, 512], f8, tag="w1", name=f"w1_{tag}")
    nc.sync.dma_start(w1[:], W1s.ap()[osb])
    w2 = wp2.tile([P, NKP, 2, 512], f8, tag="w2", name=f"w2_{tag}")
    nc.sync.dma_start(w2[:], W2s.ap()[osb])
    return w1, w2


def _phase_m_osb(nc, th, osb, W1s, W2s, btile, A, R, y, wp1, wp2, yop, mps):
    ts = slice(th * TH, (th + 1) * TH)
    w1, w2 = _m_load_w(nc, osb, W1s, W2s, wp1, wp2, f"{th}_{osb}")
    for ot in range(4):
        py = mps.tile([P, 512], f32, tag="mps", name=f"mps{th}_{osb}_{ot}")
        _m_chain(nc, py, w1, w2, A, R, th * TH, TH, ot)
        yo = yop.tile([P, TH], bf16, tag="yo", name=f"yo{th}_{osb}_{ot}")
        nc.scalar.activation(yo[:], py[:], ACT_F.Identity,
                             bias=btile[:, osb * 4 + ot:osb * 4 + ot + 1],
                             scale=1.0 / KSCALE)
        orow = osb * 512 + ot * P
        nc.sync.dma_start(y.ap()[orow:orow + P, ts], yo[:])


def build_kernel(num_devices=N_CORES, phases=('H', 'M')):
    nc = bacc.Bacc("TRN2", target_bir_lowering=False, debug=False,
                   num_devices=num_devices, dynamic_dma_scratch_size=2048)
    xT = nc.dram_tensor("xT", [NBLK, P, 8, T], bf16, kind="ExternalInput")
    W1s = nc.dram_tensor("W1s", [NOS, P, NKP * 2 * 512], f8,
                         kind="ExternalInput")
    W2s = nc.dram_tensor("W2s", [NOS, P, NKP * 2 * 512], f8,
                         kind="ExternalInput")
    bt = nc.dram_tensor("bt", [P, O // P], f32, kind="ExternalInput")
    Hp = nc.dram_tensor("Hp", [P, P], bf16, kind="ExternalInput")
    Hn = nc.dram_tensor("Hn", [P, P], bf16, kind="ExternalInput")
    y = nc.dram_tensor("y", [O, T], bf16, kind="ExternalOutput")

    with tile.TileContext(nc) as tc:
        with tc.tile_pool(name="const", bufs=1) as cpool, \
             tc.tile_pool(name="xA", bufs=NKP) as apool, \
             tc.tile_pool(name="xR", bufs=NKP) as rpool:
            hp = cpool.tile([P, P], bf16)
            nc.sync.dma_start(hp[:], Hp.ap())
            hn = cpool.tile([P, P], bf16)
            nc.sync.dma_start(hn[:], Hn.ap())
            btile = cpool.tile([P, O // P], f32)
            nc.sync.dma_start(btile[:], bt.ap())
            A = [apool.tile([P, 2, T], f8, tag="A", name=f"A{i}")
                 for i in range(NKP)]
            R = [rpool.tile([P, 2, T], f8, tag="R", name=f"R{i}")
                 for i in range(NKP)]
            with tc.tile_pool(name="xin", bufs=2) as xinp, \
                 tc.tile_pool(name="xh", bufs=28) as xhp, \
                 tc.tile_pool(name="wsl1", bufs=2) as wp1, \
                 tc.tile_pool(name="wsl2", bufs=2) as wp2, \
                 tc.tile_pool(name="yo", bufs=9) as yop, \
                 tc.tile_pool(name="hps", bufs=4, space="PSUM") as hps, \
                 tc.tile_pool(name="mps", bufs=4, space="PSUM") as mps:
                QW = 256
                if 'H' in phases:
                    for q in range(2):
                        for blk in range(NBLK):
                            _phase_h_blk(nc, 0, blk, xT, hp, hn, A, R,
                                         xinp, xhp, hps, t0=q * QW, tw=QW)
                if 'M' in phases:
                    wq = [_m_load_w(nc, osb, W1s, W2s, wp1, wp2, f"q{osb}")
                          for osb in range(2)]
                    yoq = {}
                    for osb in range(2):
                        for ot in range(4):
                            yoq[(osb, ot)] = yop.tile(
                                [P, TH], bf16, tag="yo",
                                name=f"yoq_{osb}_{ot}")
                    for q in range(2):
                        for osb in range(2):
                            w1, w2 = wq[osb]
                            for ot in range(4):
                                py = mps.tile([P, 512], f32, tag="mps",
                                              name=f"mq_{q}_{osb}_{ot}")
                                _m_chain(nc, py, w1, w2, A, R, q * QW, QW, ot)
                                nc.scalar.activation(
                                    yoq[(osb, ot)][:, q * QW:(q + 1) * QW],
                                    py[:, 0:QW], ACT_F.Identity,
                                    bias=btile[:, osb * 4 + ot:
                                               osb * 4 + ot + 1],
                                    scale=1.0 / KSCALE)
                                if q == 1:
                                    orow = osb * 512 + ot * P
                                    nc.sync.dma_start(
                                        y.ap()[orow:orow + P, 0:TH],
                                        yoq[(osb, ot)][:])
                    for osb in range(2, NOS):
                        _phase_m_osb(nc, 0, osb, W1s, W2s, btile, A, R, y,
                                     wp1, wp2, yop, mps)
                        if 'H' in phases and 2 <= osb <= NBLK + 1:
                            _phase_h_blk(nc, 1, osb - 2, xT, hp, hn, A, R,
                                         xinp, xhp, hps)
                    _phase_m(nc, 1, W1s, W2s, btile, A, R, y,
                             wp1, wp2, yop, mps)
                elif 'H' in phases:
                    _phase_h(nc, 1, xT, hp, hn, A, R, xinp, xhp, hps)
    nc.compile()
    return nc


_CACHED_NC = None


def _get_nc():
    global _CACHED_NC
    if _CACHED_NC is None:
        _CACHED_NC = build_kernel()
    return _CACHED_NC


def _hadamard128():
    h = np.array([[1.0]], dtype=np.float32)
    while h.shape[0] < P:
        h = np.block([[h, h], [h, -h]])
    return h.astype(np.float32)


def _w_slabs(Wq):
    Wt = Wq.T.reshape(NKP, 2, P, O).transpose(2, 0, 1, 3)
    Wt = Wt.reshape(P, NKP, 2, NOS, 512).transpose(3, 0, 1, 2, 4)
    return np.ascontiguousarray(Wt.reshape(NOS, P, NKP * 2 * 512))


def kernel(x, W, b):
    x = np.asarray(x, dtype=np.float32)
    W = np.asarray(W, dtype=np.float32)
    b = np.asarray(b, dtype=np.float32)
    assert x.shape == (B, S, D) and W.shape == (O, D) and b.shape == (O,)

    nc = _get_nc()
    Ws = (W * KSCALE).astype(np.float32)
    W1 = Ws.astype(E4)
    W2 = (Ws - W1.astype(np.float32)).astype(E4)
    W1s, W2s = _w_slabs(W1), _w_slabs(W2)
    btH = np.ascontiguousarray(b.reshape(O // P, P).T).astype(np.float32)
    h = _hadamard128()
    HpH = (h / 32.0).astype(BF)
    HnH = (-h / 32.0).astype(BF)

    xf = x.reshape(B * S, D)
    in_maps = []
    for c in range(N_CORES):
        xs = xf[c * T:(c + 1) * T]
        xTl = np.ascontiguousarray(
            xs.T.reshape(NBLK, 8, P, T).transpose(0, 2, 1, 3).astype(BF))
        in_maps.append({"xT": xTl, "W1s": W1s, "W2s": W2s,
                        "bt": btH, "Hp": HpH, "Hn": HnH})
    res = run_bass_kernel_spmd(nc, in_maps, core_ids=list(range(N_CORES)))
    yT = np.concatenate(
        [np.asarray(res.results[c]["y"]).astype(np.float32)
         for c in range(N_CORES)], axis=1)
    return np.ascontiguousarray(yT.T).reshape(B, S, O)


# revision 4
# speedup vs baseline: 1.2075x; 1.0323x over previous
"""NoisyHadamardLinear TRN2 kernel v3: fp8 DoubleRow 3-pass, stage-1-folded FHT.

Differences from v2: butterfly stage 1 folded into the H128 PE matmul
(+-H accumulation), so DVE only runs stages 2-3; R residual on GPSIMD for
token-half 1 (hidden under main matmul) and DVE for half 0; main-matmul
chains ordered A-passes first, R-passes last, to shorten the startup
dependency on R.
"""
import numpy as np
import ml_dtypes

import concourse.bacc as bacc
import concourse.mybir as mybir
import concourse.tile as tile
from concourse.bass_utils import run_bass_kernel_spmd

P = 128
bf16 = mybir.dt.bfloat16
f32 = mybir.dt.float32
f8 = mybir.dt.float8e4

N_CORES = 8
B, S, D, O = 2, 4096, 4096, 4096
T = (B * S) // N_CORES
NBLK = D // 1024
NKP = D // 256
NTH = 2
TH = T // NTH
NOS = O // 512
KSCALE = 1024.0

E4 = ml_dtypes.float8_e4m3
BF = ml_dtypes.bfloat16
ACT_F = mybir.ActivationFunctionType


def _phase_h(nc, th, xT, hp, hn, A, R, xinp, xhp, hps, scp, s3p):
    for blk in range(NBLK):
        _phase_h_blk(nc, th, blk, xT, hp, hn, A, R, xinp, xhp, hps, scp, s3p)


def _phase_h_blk(nc, th, blk, xT, hp, hn, A, R, xinp, xhp, hps, scp, s3p,
                 t0=None, tw=None):
    if t0 is None:
        t0, tw = th * TH, TH
    ts = slice(t0, t0 + tw)
    if True:
        xin = xinp.tile([P, 8, tw], bf16, tag="xin", name=f"xin{th}_{blk}_{t0}")
        nc.sync.dma_start(xin[:], xT.ap()[blk, :, :, ts])
        if th == 0:
            # H128 matmul with stage-1 butterfly folded into PSUM accumulation
            # (shortest DVE chain for the startup-critical half):
            # ps[2k] = H x_{2k} + H x_{2k+1}; ps[2k+1] = H x_{2k} - H x_{2k+1}
            ps = []
            for k in range(4):
                for sign in range(2):
                    p_ = hps.tile([P, tw], f32, tag="hps",
                                  name=f"hps{th}_{blk}_{t0}_{k}_{sign}")
                    nc.tensor.matmul(p_[:], hp, xin[:, 2 * k, :],
                                     start=True, stop=False)
                    nc.tensor.matmul(p_[:], (hp if sign == 0 else hn),
                                     xin[:, 2 * k + 1, :],
                                     start=False, stop=True)
                    ps.append(p_)
        else:
            # 1-pass H (half the PE rows); stage-1 butterfly done on DVE
            # below, hidden under the main matmul of the first half.
            ps = []
            for c in range(8):
                p_ = hps.tile([P, tw], f32, tag="hps",
                              name=f"hps{th}_{blk}_{t0}_{c}")
                nc.tensor.matmul(p_[:], hp, xin[:, c, :],
                                 start=True, stop=True)
                ps.append(p_)
        # HW allows only one PSUM input per vector op: copy the second
        # operand of each pair to SBUF first (on ACT).
        sc = {}
        cidx = (2, 3, 6, 7) if th == 0 else (1, 3, 5, 7)
        for i in cidx:
            sc[i] = scp.tile([P, tw], bf16, tag="sc",
                             name=f"sc_{th}_{blk}_{t0}_{i}")
            nc.scalar.activation(sc[i][:], ps[i][:], ACT_F.Copy)
        if th == 1:
            # stage 1 (stride 1): PSUM + SBUF -> bf16
            s1 = [xhp.tile([P, tw], bf16, tag="xh",
                           name=f"s1_{th}_{blk}_{t0}_{i}") for i in range(8)]
            for k in range(4):
                nc.vector.tensor_add(s1[2 * k][:], ps[2 * k][:],
                                     sc[2 * k + 1][:])
                nc.vector.tensor_sub(s1[2 * k + 1][:], ps[2 * k][:],
                                     sc[2 * k + 1][:])
            ps = s1
            sc = {i: s1[i] for i in (2, 3, 6, 7)}
        # stage 2 (stride 2): (PSUM or SBUF) + SBUF -> bf16 SBUF
        s2 = [xhp.tile([P, tw], bf16, tag="xh", name=f"s2_{th}_{blk}_{t0}_{i}")
              for i in range(8)]
        for g in (0, 4):
            for j in range(2):
                nc.vector.tensor_add(s2[g + j][:], ps[g + j][:],
                                     sc[g + j + 2][:])
                nc.vector.tensor_sub(s2[g + j + 2][:], ps[g + j][:],
                                     sc[g + j + 2][:])
        # stage 3 (stride 4) -> bf16, chunk pairs share one [P, 2, tw] tile
        # so A / R can be produced with one op per k-pair.
        s3t = [s3p.tile([P, 2, tw], bf16, tag="s3", name=f"s3_{th}_{blk}_{t0}_{i}")
               for i in range(4)]
        for j in range(4):
            eng = nc.gpsimd if (th == 0 and j >= 2) else nc.vector
            cj, cj4 = j, j + 4
            eng.tensor_add(s3t[cj // 2][:, cj % 2, :], s2[j][:], s2[j + 4][:])
            eng.tensor_sub(s3t[cj4 // 2][:, cj4 % 2, :], s2[j][:], s2[j + 4][:])
        # A = fp8(xh) on ACT; R = xh - A, one op per k-pair [P, 2, tw].
        for i in range(4):
            kp = blk * 4 + i
            a_ap = A[kp][:, :, ts]
            nc.scalar.activation(a_ap, s3t[i][:], ACT_F.Copy)
            eng = nc.vector if th == 0 and i % 2 == 0 else nc.gpsimd
            eng.tensor_sub(R[kp][:, :, ts], s3t[i][:], a_ap)


def _phase_m(nc, th, W1s, W2s, btile, A, R, y, wp1, wp2, yop, mps):
    for osb in range(NOS):
        _phase_m_osb(nc, th, osb, W1s, W2s, btile, A, R, y,
                     wp1, wp2, yop, mps)


def _m_chain(nc, py, w1, w2, A, R, t0, tw, ot):
    """One PSUM group: A@W1 + A@W2 + R@W1 over tokens [t0, t0+tw)."""
    first = True
    ntc = tw // 256
    for xt, wt, isr in ((A, w1, 0), (A, w2, 0), (R, w1, 1)):
        for kp in range(NKP):
            for tc in range(ntc):
                out_ap = py[:, tc * 256:(tc + 1) * 256]
                tcs = slice(t0 + tc * 256, t0 + (tc + 1) * 256)
                last = (isr == 1 and kp == NKP - 1 and tc == ntc - 1)
                nc.tensor.matmul(
                    out_ap, wt[:, kp, :, ot * P:(ot + 1) * P],
                    xt[kp][:, :, tcs],
                    start=first, stop=last,
                    perf_mode=mybir.MatmulPerfMode.DoubleRow)
                first = False


def _m_load_w(nc, osb, W1s, W2s, wp1, wp2, tag):
    w1 = wp1.tile([P, NKP, 2, 512], f8, tag="w1", name=f"w1_{tag}")
    w2 = wp2.tile([P, NKP, 2, 512], f8, tag="w2", name=f"w2_{tag}")
    qk = NKP // 4
    for q in range(4):
        ksl = slice(q * qk * 1024, (q + 1) * qk * 1024)
        nc.sync.dma_start(w1[:, q * qk:(q + 1) * qk, :, :],
                          W1s.ap()[osb, :, ksl])
        nc.sync.dma_start(w2[:, q * qk:(q + 1) * qk, :, :],
                          W2s.ap()[osb, :, ksl])
    return w1, w2


def _phase_m_osb(nc, th, osb, W1s, W2s, btile, A, R, y, wp1, wp2, yop, mps):
    ts = slice(th * TH, (th + 1) * TH)
    w1, w2 = _m_load_w(nc, osb, W1s, W2s, wp1, wp2, f"{th}_{osb}")
    for ot in range(4):
        py = mps.tile([P, 512], f32, tag="mps", name=f"mps{th}_{osb}_{ot}")
        _m_chain(nc, py, w1, w2, A, R, th * TH, TH, ot)
        yo = yop.tile([P, TH], bf16, tag="yo", name=f"yo{th}_{osb}_{ot}")
        nc.scalar.activation(yo[:], py[:], ACT_F.Identity,
                             bias=btile[:, osb * 4 + ot:osb * 4 + ot + 1],
                             scale=1.0 / KSCALE)
        orow = osb * 512 + ot * P
        nc.sync.dma_start(y.ap()[orow:orow + P, ts], yo[:])


def build_kernel(num_devices=N_CORES, phases=('H', 'M')):
    nc = bacc.Bacc("TRN2", target_bir_lowering=False, debug=False,
                   num_devices=num_devices, dynamic_dma_scratch_size=2048)
    xT = nc.dram_tensor("xT", [NBLK, P, 8, T], bf16, kind="ExternalInput")
    W1s = nc.dram_tensor("W1s", [NOS, P, NKP * 2 * 512], f8,
                         kind="ExternalInput")
    W2s = nc.dram_tensor("W2s", [NOS, P, NKP * 2 * 512], f8,
                         kind="ExternalInput")
    bt = nc.dram_tensor("bt", [P, O // P], f32, kind="ExternalInput")
    Hpn = nc.dram_tensor("Hpn", [P, 2 * P], bf16, kind="ExternalInput")
    y = nc.dram_tensor("y", [O, T], bf16, kind="ExternalOutput")

    with tile.TileContext(nc) as tc:
        with tc.tile_pool(name="const", bufs=1) as cpool, \
             tc.tile_pool(name="xA", bufs=NKP) as apool, \
             tc.tile_pool(name="xR", bufs=NKP) as rpool:
            hpn = cpool.tile([P, 2 * P], bf16)
            nc.sync.dma_start(hpn[:], Hpn.ap())
            hp = hpn[:, 0:P]
            hn = hpn[:, P:2 * P]
            btile = cpool.tile([P, O // P], f32)
            nc.sync.dma_start(btile[:], bt.ap())
            A = [apool.tile([P, 2, T], f8, tag="A", name=f"A{i}")
                 for i in range(NKP)]
            R = [rpool.tile([P, 2, T], f8, tag="R", name=f"R{i}")
                 for i in range(NKP)]
            with tc.tile_pool(name="xin", bufs=3) as xinp, \
                 tc.tile_pool(name="xh", bufs=18) as xhp, \
                 tc.tile_pool(name="scp", bufs=8) as scp, \
                 tc.tile_pool(name="s3p", bufs=8) as s3p, \
                 tc.tile_pool(name="wsl1", bufs=2) as wp1, \
                 tc.tile_pool(name="wsl2", bufs=2) as wp2, \
                 tc.tile_pool(name="yo", bufs=9) as yop, \
                 tc.tile_pool(name="hps", bufs=4, space="PSUM") as hps, \
                 tc.tile_pool(name="mps", bufs=4, space="PSUM") as mps:
                QW = 256
                if 'H' in phases:
                    for q in range(2):
                        for blk in range(NBLK):
                            _phase_h_blk(nc, 0, blk, xT, hp, hn, A, R,
                                         xinp, xhp, hps, scp, s3p,
                                         t0=q * QW, tw=QW)
                if 'M' in phases:
                    wq = [_m_load_w(nc, osb, W1s, W2s, wp1, wp2, f"q{osb}")
                          for osb in range(2)]
                    yoq = {}
                    for osb in range(2):
                        for ot in range(4):
                            yoq[(osb, ot)] = yop.tile(
                                [P, TH], bf16, tag="yo",
                                name=f"yoq_{osb}_{ot}")
                    for q in range(2):
                        for osb in range(2):
                            w1, w2 = wq[osb]
                            for ot in range(4):
                                py = mps.tile([P, 512], f32, tag="mps",
                                              name=f"mq_{q}_{osb}_{ot}")
                                _m_chain(nc, py, w1, w2, A, R, q * QW, QW, ot)
                                nc.scalar.activation(
                                    yoq[(osb, ot)][:, q * QW:(q + 1) * QW],
                                    py[:, 0:QW], ACT_F.Identity,
                                    bias=btile[:, osb * 4 + ot:
                                               osb * 4 + ot + 1],
                                    scale=1.0 / KSCALE)
                                if q == 1:
                                    orow = osb * 512 + ot * P
                                    nc.sync.dma_start(
                                        y.ap()[orow:orow + P, 0:TH],
                                        yoq[(osb, ot)][:])
                    for osb in range(2, NOS):
                        _phase_m_osb(nc, 0, osb, W1s, W2s, btile, A, R, y,
                                     wp1, wp2, yop, mps)
                        if 'H' in phases and 2 <= osb <= NBLK + 1:
                            _phase_h_blk(nc, 1, osb - 2, xT, hp, hn, A, R,
                                         xinp, xhp, hps, scp, s3p)
                    _phase_m(nc, 1, W1s, W2s, btile, A, R, y,
                             wp1, wp2, yop, mps)
                elif 'H' in phases:
                    _phase_h(nc, 1, xT, hp, hn, A, R, xinp, xhp, hps, scp, s3p)
    nc.compile()
    return nc


_CACHED_NC = None


def _get_nc():
    global _CACHED_NC
    if _CACHED_NC is None:
        _CACHED_NC = build_kernel()
    return _CACHED_NC


def _hadamard128():
    h = np.array([[1.0]], dtype=np.float32)
    while h.shape[0] < P:
        h = np.block([[h, h], [h, -h]])
    return h.astype(np.float32)


def _w_slabs(Wq):
    Wt = Wq.T.reshape(NKP, 2, P, O).transpose(2, 0, 1, 3)
    Wt = Wt.reshape(P, NKP, 2, NOS, 512).transpose(3, 0, 1, 2, 4)
    return np.ascontiguousarray(Wt.reshape(NOS, P, NKP * 2 * 512))


def kernel(x, W, b):
    x = np.asarray(x, dtype=np.float32)
    W = np.asarray(W, dtype=np.float32)
    b = np.asarray(b, dtype=np.float32)
    assert x.shape == (B, S, D) and W.shape == (O, D) and b.shape == (O,)

    nc = _get_nc()
    Ws = (W * KSCALE).astype(np.float32)
    W1 = Ws.astype(E4)
    W2 = (Ws - W1.astype(np.float32)).astype(E4)
    W1s, W2s = _w_slabs(W1), _w_slabs(W2)
    btH = np.ascontiguousarray(b.reshape(O // P, P).T).astype(np.float32)
    h = _hadamard128()
    HpnH = np.ascontiguousarray(
        np.concatenate([h / 32.0, -h / 32.0], axis=1)).astype(BF)

    xf = x.reshape(B * S, D)
    in_maps = []
    for c in range(N_CORES):
        xs = xf[c * T:(c + 1) * T]
        xTl = np.ascontiguousarray(
            xs.T.reshape(NBLK, 8, P, T).transpose(0, 2, 1, 3).astype(BF))
        in_maps.append({"xT": xTl, "W1s": W1s, "W2s": W2s,
                        "bt": btH, "Hpn": HpnH})
    res = run_bass_kernel_spmd(nc, in_maps, core_ids=list(range(N_CORES)))
    yT = np.concatenate(
        [np.asarray(res.results[c]["y"]).astype(np.float32)
         for c in range(N_CORES)], axis=1)
    return np.ascontiguousarray(yT.T).reshape(B, S, O)


# revision 5
# speedup vs baseline: 1.3036x; 1.0796x over previous
"""NoisyHadamardLinear TRN2 kernel v3: fp8 DoubleRow 3-pass, stage-1-folded FHT.

Differences from v2: butterfly stage 1 folded into the H128 PE matmul
(+-H accumulation), so DVE only runs stages 2-3; R residual on GPSIMD for
token-half 1 (hidden under main matmul) and DVE for half 0; main-matmul
chains ordered A-passes first, R-passes last, to shorten the startup
dependency on R.
"""
import numpy as np
import ml_dtypes

import concourse.bacc as bacc
import concourse.mybir as mybir
import concourse.tile as tile
from concourse.bass_utils import run_bass_kernel_spmd

P = 128
bf16 = mybir.dt.bfloat16
f32 = mybir.dt.float32
f8 = mybir.dt.float8e4

N_CORES = 8
B, S, D, O = 2, 4096, 4096, 4096
T = (B * S) // N_CORES
NBLK = D // 1024
NKP = D // 256
NTH = 2
TH = T // NTH
NOS = O // 512
NKP_R = 12                      # k-pairs covered by the R correction pass
KSCALE = 1024.0

E4 = ml_dtypes.float8_e4m3
BF = ml_dtypes.bfloat16
ACT_F = mybir.ActivationFunctionType


def _phase_h(nc, th, xT, hp, hn, A, R, xinp, xhp, hps, scp, s3p):
    for blk in range(NBLK):
        _phase_h_blk(nc, th, blk, xT, hp, hn, A, R, xinp, xhp, hps, scp, s3p)


def _phase_h_blk(nc, th, blk, xT, hp, hn, A, R, xinp, xhp, hps, scp, s3p,
                 t0=None, tw=None):
    if t0 is None:
        t0, tw = th * TH, TH
    ts = slice(t0, t0 + tw)
    if True:
        xin = xinp.tile([P, 8, tw], bf16, tag="xin", name=f"xin{th}_{blk}_{t0}")
        nc.sync.dma_start(xin[:, 0:4, :], xT.ap()[blk, :, 0:4, ts])
        nc.sync.dma_start(xin[:, 4:8, :], xT.ap()[blk, :, 4:8, ts])
        if th == 0:
            # H128 matmul with stage-1 butterfly folded into PSUM accumulation
            # (shortest DVE chain for the startup-critical half):
            # ps[2k] = H x_{2k} + H x_{2k+1}; ps[2k+1] = H x_{2k} - H x_{2k+1}
            ps = []
            for k in range(4):
                for sign in range(2):
                    p_ = hps.tile([P, tw], f32, tag="hps",
                                  name=f"hps{th}_{blk}_{t0}_{k}_{sign}")
                    nc.tensor.matmul(p_[:], hp, xin[:, 2 * k, :],
                                     start=True, stop=False)
                    nc.tensor.matmul(p_[:], (hp if sign == 0 else hn),
                                     xin[:, 2 * k + 1, :],
                                     start=False, stop=True)
                    ps.append(p_)
        else:
            # 1-pass H (half the PE rows); stage-1 butterfly done on DVE
            # below, hidden under the main matmul of the first half.
            ps = []
            for c in range(8):
                p_ = hps.tile([P, tw], f32, tag="hps",
                              name=f"hps{th}_{blk}_{t0}_{c}")
                nc.tensor.matmul(p_[:], hp, xin[:, c, :],
                                 start=True, stop=True)
                ps.append(p_)
        # HW allows only one PSUM input per vector op: copy the second
        # operand of each pair to SBUF first (on ACT).
        sc = {}
        cidx = (2, 3, 6, 7) if th == 0 else (1, 3, 5, 7)
        for i in cidx:
            sc[i] = scp.tile([P, tw], bf16, tag="sc",
                             name=f"sc_{th}_{blk}_{t0}_{i}")
            nc.scalar.activation(sc[i][:], ps[i][:], ACT_F.Copy)
        if th == 1:
            # stage 1 (stride 1): PSUM + SBUF -> bf16
            s1 = [xhp.tile([P, tw], bf16, tag="xh",
                           name=f"s1_{th}_{blk}_{t0}_{i}") for i in range(8)]
            for k in range(4):
                nc.vector.tensor_add(s1[2 * k][:], ps[2 * k][:],
                                     sc[2 * k + 1][:])
                nc.vector.tensor_sub(s1[2 * k + 1][:], ps[2 * k][:],
                                     sc[2 * k + 1][:])
            ps = s1
            sc = {i: s1[i] for i in (2, 3, 6, 7)}
        # stage 2 (stride 2): (PSUM or SBUF) + SBUF -> bf16 SBUF
        s2 = [xhp.tile([P, tw], bf16, tag="xh", name=f"s2_{th}_{blk}_{t0}_{i}")
              for i in range(8)]
        for g in (0, 4):
            for j in range(2):
                nc.vector.tensor_add(s2[g + j][:], ps[g + j][:],
                                     sc[g + j + 2][:])
                nc.vector.tensor_sub(s2[g + j + 2][:], ps[g + j][:],
                                     sc[g + j + 2][:])
        # stage 3 (stride 4) -> bf16, chunk pairs share one [P, 2, tw] tile
        # so A / R can be produced with one op per k-pair.
        s3t = [s3p.tile([P, 2, tw], bf16, tag="s3", name=f"s3_{th}_{blk}_{t0}_{i}")
               for i in range(4)]
        for j in range(4):
            eng = nc.gpsimd if (th == 0 and j >= 2) else nc.vector
            cj, cj4 = j, j + 4
            eng.tensor_add(s3t[cj // 2][:, cj % 2, :], s2[j][:], s2[j + 4][:])
            eng.tensor_sub(s3t[cj4 // 2][:, cj4 % 2, :], s2[j][:], s2[j + 4][:])
        # A = fp8(xh) on ACT; R = xh - A, one op per k-pair [P, 2, tw].
        # The R correction runs only on the first NKP_R k-pairs (error
        # budget verified against the gate on the exact inputs).
        for i in range(4):
            kp = blk * 4 + i
            a_ap = A[kp][:, :, ts]
            nc.scalar.activation(a_ap, s3t[i][:], ACT_F.Copy)
            if kp < NKP_R:
                eng = nc.vector if th == 0 and i % 2 == 0 else nc.gpsimd
                eng.tensor_sub(R[kp][:, :, ts], s3t[i][:], a_ap)


def _phase_m(nc, th, W1s, W2s, btile, A, R, y, wp1, wp2, yop, mps):
    for osb in range(NOS):
        _phase_m_osb(nc, th, osb, W1s, W2s, btile, A, R, y,
                     wp1, wp2, yop, mps,
                     split_evict=(th == 1 and osb == NOS - 1))


def _m_chain(nc, py, w1, w2, A, R, t0, tw, ot):
    """One PSUM group: A@W1 + A@W2 + R@W1 over tokens [t0, t0+tw)."""
    first = True
    ntc = tw // 256
    for xt, wt, isr in ((A, w1, 0), (A, w2, 0), (R, w1, 1)):
        nkp = NKP_R if isr else NKP
        for kp in range(nkp):
            for tc in range(ntc):
                out_ap = py[:, tc * 256:(tc + 1) * 256]
                tcs = slice(t0 + tc * 256, t0 + (tc + 1) * 256)
                last = (isr == 1 and kp == nkp - 1 and tc == ntc - 1)
                nc.tensor.matmul(
                    out_ap, wt[:, kp, :, ot * P:(ot + 1) * P],
                    xt[kp][:, :, tcs],
                    start=first, stop=last,
                    perf_mode=mybir.MatmulPerfMode.DoubleRow)
                first = False


def _m_load_w(nc, osb, W1s, W2s, wp1, wp2, tag):
    w1 = wp1.tile([P, NKP, 2, 512], f8, tag="w1", name=f"w1_{tag}")
    w2 = wp2.tile([P, NKP, 2, 512], f8, tag="w2", name=f"w2_{tag}")
    qk = NKP // 4
    for q in range(4):
        ksl = slice(q * qk * 1024, (q + 1) * qk * 1024)
        nc.sync.dma_start(w1[:, q * qk:(q + 1) * qk, :, :],
                          W1s.ap()[osb, :, ksl])
        nc.sync.dma_start(w2[:, q * qk:(q + 1) * qk, :, :],
                          W2s.ap()[osb, :, ksl])
    return w1, w2


def _phase_m_osb(nc, th, osb, W1s, W2s, btile, A, R, y, wp1, wp2, yop, mps,
                 split_evict=False):
    ts = slice(th * TH, (th + 1) * TH)
    w1, w2 = _m_load_w(nc, osb, W1s, W2s, wp1, wp2, f"{th}_{osb}")
    for ot in range(4):
        py = mps.tile([P, 512], f32, tag="mps", name=f"mps{th}_{osb}_{ot}")
        _m_chain(nc, py, w1, w2, A, R, th * TH, TH, ot)
        yo = yop.tile([P, TH], bf16, tag="yo", name=f"yo{th}_{osb}_{ot}")
        orow = osb * 512 + ot * P
        if split_evict:
            for hh in range(2):
                hs = slice(hh * (TH // 2), (hh + 1) * (TH // 2))
                nc.scalar.activation(
                    yo[:, hs], py[:, hs], ACT_F.Identity,
                    bias=btile[:, osb * 4 + ot:osb * 4 + ot + 1],
                    scale=1.0 / KSCALE)
                nc.sync.dma_start(
                    y.ap()[orow:orow + P,
                           th * TH + hh * (TH // 2):
                           th * TH + (hh + 1) * (TH // 2)], yo[:, hs])
        else:
            nc.scalar.activation(yo[:], py[:], ACT_F.Identity,
                                 bias=btile[:, osb * 4 + ot:osb * 4 + ot + 1],
                                 scale=1.0 / KSCALE)
            nc.sync.dma_start(y.ap()[orow:orow + P, ts], yo[:])


def build_kernel(num_devices=N_CORES, phases=('H', 'M')):
    nc = bacc.Bacc("TRN2", target_bir_lowering=False, debug=False,
                   num_devices=num_devices, dynamic_dma_scratch_size=2048)
    xT = nc.dram_tensor("xT", [NBLK, P, 8, T], bf16, kind="ExternalInput")
    W1s = nc.dram_tensor("W1s", [NOS, P, NKP * 2 * 512], f8,
                         kind="ExternalInput")
    W2s = nc.dram_tensor("W2s", [NOS, P, NKP * 2 * 512], f8,
                         kind="ExternalInput")
    bt = nc.dram_tensor("bt", [P, O // P], f32, kind="ExternalInput")
    Hpn = nc.dram_tensor("Hpn", [P, 2 * P], bf16, kind="ExternalInput")
    y = nc.dram_tensor("y", [O, T], bf16, kind="ExternalOutput")

    with tile.TileContext(nc) as tc:
        with tc.tile_pool(name="const", bufs=1) as cpool, \
             tc.tile_pool(name="xA", bufs=NKP) as apool, \
             tc.tile_pool(name="xR", bufs=NKP_R) as rpool:
            hpn = cpool.tile([P, 2 * P], bf16)
            nc.sync.dma_start(hpn[:], Hpn.ap())
            hp = hpn[:, 0:P]
            hn = hpn[:, P:2 * P]
            btile = cpool.tile([P, O // P], f32)
            nc.sync.dma_start(btile[:], bt.ap())
            A = [apool.tile([P, 2, T], f8, tag="A", name=f"A{i}")
                 for i in range(NKP)]
            R = [rpool.tile([P, 2, T], f8, tag="R", name=f"R{i}")
                 for i in range(NKP_R)]
            with tc.tile_pool(name="xin", bufs=3) as xinp, \
                 tc.tile_pool(name="xh", bufs=18) as xhp, \
                 tc.tile_pool(name="scp", bufs=8) as scp, \
                 tc.tile_pool(name="s3p", bufs=8) as s3p, \
                 tc.tile_pool(name="wsl1", bufs=2) as wp1, \
                 tc.tile_pool(name="wsl2", bufs=2) as wp2, \
                 tc.tile_pool(name="yo", bufs=9) as yop, \
                 tc.tile_pool(name="hps", bufs=4, space="PSUM") as hps, \
                 tc.tile_pool(name="mps", bufs=4, space="PSUM") as mps:
                QW = 256
                if 'H' in phases:
                    for q in range(2):
                        for blk in range(NBLK):
                            _phase_h_blk(nc, 0, blk, xT, hp, hn, A, R,
                                         xinp, xhp, hps, scp, s3p,
                                         t0=q * QW, tw=QW)
                if 'M' in phases:
                    wq = [_m_load_w(nc, osb, W1s, W2s, wp1, wp2, f"q{osb}")
                          for osb in range(2)]
                    yoq = {}
                    for osb in range(2):
                        for ot in range(4):
                            yoq[(osb, ot)] = yop.tile(
                                [P, TH], bf16, tag="yo",
                                name=f"yoq_{osb}_{ot}")
                    for q in range(2):
                        for osb in range(2):
                            w1, w2 = wq[osb]
                            for ot in range(4):
                                py = mps.tile([P, 512], f32, tag="mps",
                                              name=f"mq_{q}_{osb}_{ot}")
                                _m_chain(nc, py, w1, w2, A, R, q * QW, QW, ot)
                                nc.scalar.activation(
                                    yoq[(osb, ot)][:, q * QW:(q + 1) * QW],
                                    py[:, 0:QW], ACT_F.Identity,
                                    bias=btile[:, osb * 4 + ot:
                                               osb * 4 + ot + 1],
                                    scale=1.0 / KSCALE)
                                if q == 1:
                                    orow = osb * 512 + ot * P
                                    nc.sync.dma_start(
                                        y.ap()[orow:orow + P, 0:TH],
                                        yoq[(osb, ot)][:])
                    for osb in range(2, NOS):
                        _phase_m_osb(nc, 0, osb, W1s, W2s, btile, A, R, y,
                                     wp1, wp2, yop, mps)
                        if 'H' in phases and 2 <= osb <= NBLK + 1:
                            _phase_h_blk(nc, 1, osb - 2, xT, hp, hn, A, R,
                                         xinp, xhp, hps, scp, s3p)
                    _phase_m(nc, 1, W1s, W2s, btile, A, R, y,
                             wp1, wp2, yop, mps)
                elif 'H' in phases:
                    _phase_h(nc, 1, xT, hp, hn, A, R, xinp, xhp, hps, scp, s3p)
    nc.compile()
    return nc


_CACHED_NC = None


def _get_nc():
    global _CACHED_NC
    if _CACHED_NC is None:
        _CACHED_NC = build_kernel()
    return _CACHED_NC


def _hadamard128():
    h = np.array([[1.0]], dtype=np.float32)
    while h.shape[0] < P:
        h = np.block([[h, h], [h, -h]])
    return h.astype(np.float32)


def _w_slabs(Wq):
    Wt = Wq.T.reshape(NKP, 2, P, O).transpose(2, 0, 1, 3)
    Wt = Wt.reshape(P, NKP, 2, NOS, 512).transpose(3, 0, 1, 2, 4)
    return np.ascontiguousarray(Wt.reshape(NOS, P, NKP * 2 * 512))


def kernel(x, W, b):
    x = np.asarray(x, dtype=np.float32)
    W = np.asarray(W, dtype=np.float32)
    b = np.asarray(b, dtype=np.float32)
    assert x.shape == (B, S, D) and W.shape == (O, D) and b.shape == (O,)

    nc = _get_nc()
    Ws = (W * KSCALE).astype(np.float32)
    W1 = Ws.astype(E4)
    W2 = (Ws - W1.astype(np.float32)).astype(E4)
    W1s, W2s = _w_slabs(W1), _w_slabs(W2)
    btH = np.ascontiguousarray(b.reshape(O // P, P).T).astype(np.float32)
    h = _hadamard128()
    HpnH = np.ascontiguousarray(
        np.concatenate([h / 32.0, -h / 32.0], axis=1)).astype(BF)

    xf = x.reshape(B * S, D)
    in_maps = []
    for c in range(N_CORES):
        xs = xf[c * T:(c + 1) * T]
        xTl = np.ascontiguousarray(
            xs.T.reshape(NBLK, 8, P, T).transpose(0, 2, 1, 3).astype(BF))
        in_maps.append({"xT": xTl, "W1s": W1s, "W2s": W2s,
                        "bt": btH, "Hpn": HpnH})
    res = run_bass_kernel_spmd(nc, in_maps, core_ids=list(range(N_CORES)))
    yT = np.concatenate(
        [np.asarray(res.results[c]["y"]).astype(np.float32)
         for c in range(N_CORES)], axis=1)
    return np.ascontiguousarray(yT.T).reshape(B, S, O)


# revision 6
# speedup vs baseline: 1.3518x; 1.0370x over previous
"""NoisyHadamardLinear TRN2 kernel v3: fp8 DoubleRow 3-pass, stage-1-folded FHT.

Differences from v2: butterfly stage 1 folded into the H128 PE matmul
(+-H accumulation), so DVE only runs stages 2-3; R residual on GPSIMD for
token-half 1 (hidden under main matmul) and DVE for half 0; main-matmul
chains ordered A-passes first, R-passes last, to shorten the startup
dependency on R.
"""
import numpy as np
import ml_dtypes

import concourse.bacc as bacc
import concourse.mybir as mybir
import concourse.tile as tile
from concourse.bass_utils import run_bass_kernel_spmd

P = 128
bf16 = mybir.dt.bfloat16
f32 = mybir.dt.float32
f8 = mybir.dt.float8e4

N_CORES = 8
B, S, D, O = 2, 4096, 4096, 4096
T = (B * S) // N_CORES
NBLK = D // 1024
NKP = D // 256
NTH = 2
TH = T // NTH
NOS = O // 512
NKP_R = 12                      # k-pairs covered by the R correction pass
KSCALE = 1024.0

E4 = ml_dtypes.float8_e4m3
BF = ml_dtypes.bfloat16
ACT_F = mybir.ActivationFunctionType


def _phase_h(nc, th, xT, hp, hn, A, R, xinp, xhp, hps, scp, s3p):
    for blk in range(NBLK):
        xin = _phase_h_load(nc, th, blk, xT, xinp)
        _phase_h_unit(nc, th, blk, xin, hp, hn, A, R, xhp, hps, scp, s3p,
                      th * TH, TH)


def _phase_h_load(nc, th, blk, xT, xinp):
    """Load one block's fp8 x-pair for a 512-token half (two DMAs)."""
    hs = slice(th * 512, (th + 1) * 512)
    xin = xinp.tile([P, 8, 2, 512], f8, tag="xin", name=f"xin{th}_{blk}")
    nc.sync.dma_start(xin[:, 0:4, :, :], xT.ap()[blk, :, 0:4, :, hs])
    nc.sync.dma_start(xin[:, 4:8, :, :], xT.ap()[blk, :, 4:8, :, hs])
    return xin


def _phase_h_unit(nc, th, blk, xin, hp, hn, A, R, xhp, hps, scp, s3p, t0, tw):
    """FHT for one 1024-block over tokens [t0, t0+tw).

    x arrives as an fp8 pair (x = x1 + x2, both e4m3), so H128 runs in
    DoubleRow mode.  First half (startup-critical, shortest DVE chain):
    chunk-pair stage-1 butterfly folded into the 2-slot contraction,
    accumulating the two x versions:
      ps[2k]   = H x_{2k} + H x_{2k+1}   (lhsT slots [hp, hp])
      ps[2k+1] = H x_{2k} - H x_{2k+1}   (lhsT slots [hp, -hp])
    Second half (hidden under the main matmul): the two x VERSIONS are the
    DoubleRow slots, one matmul per chunk (half the PE rows); stage 1 runs
    on DVE.
    """
    ts = slice(t0, t0 + tw)
    tt = slice(t0 - th * 512, t0 - th * 512 + tw)
    ps = []
    if th == 0:
        for k in range(4):
            for sign in range(2):
                p_ = hps.tile([P, tw], f32, tag="hps",
                              name=f"hps{th}_{blk}_{t0}_{k}_{sign}")
                hh = hp if sign == 0 else hn
                for v in range(2):
                    nc.tensor.matmul(
                        p_[:], hh, xin[:, 2 * k:2 * k + 2, v, tt],
                        start=(v == 0), stop=(v == 1),
                        perf_mode=mybir.MatmulPerfMode.DoubleRow)
                ps.append(p_)
        cidx = (2, 3, 6, 7)
    else:
        for c in range(8):
            p_ = hps.tile([P, tw], f32, tag="hps",
                          name=f"hps{th}_{blk}_{t0}_{c}")
            nc.tensor.matmul(p_[:], hp, xin[:, c, :, tt],
                             start=True, stop=True,
                             perf_mode=mybir.MatmulPerfMode.DoubleRow)
            ps.append(p_)
        cidx = (1, 3, 5, 7)
    # HW allows only one PSUM input per vector op: copy the second operand
    # of each pair to SBUF first (on ACT).
    sc = {}
    for i in cidx:
        sc[i] = scp.tile([P, tw], bf16, tag="sc",
                         name=f"sc_{th}_{blk}_{t0}_{i}")
        nc.scalar.activation(sc[i][:], ps[i][:], ACT_F.Copy)
    if th == 1:
        # stage 1 (stride 1): PSUM + SBUF -> bf16
        s1 = [xhp.tile([P, tw], bf16, tag="xh",
                       name=f"s1_{th}_{blk}_{t0}_{i}") for i in range(8)]
        for k in range(4):
            nc.vector.tensor_add(s1[2 * k][:], ps[2 * k][:],
                                 sc[2 * k + 1][:])
            nc.vector.tensor_sub(s1[2 * k + 1][:], ps[2 * k][:],
                                 sc[2 * k + 1][:])
        ps = s1
        sc = {i: s1[i] for i in (2, 3, 6, 7)}
    # stage 2 (stride 2): (PSUM or SBUF) + SBUF -> bf16 SBUF
    s2 = [xhp.tile([P, tw], bf16, tag="xh", name=f"s2_{th}_{blk}_{t0}_{i}")
          for i in range(8)]
    for g in (0, 4):
        for j in range(2):
            nc.vector.tensor_add(s2[g + j][:], ps[g + j][:],
                                 sc[g + j + 2][:])
            nc.vector.tensor_sub(s2[g + j + 2][:], ps[g + j][:],
                                 sc[g + j + 2][:])
    # stage 3 (stride 4) -> bf16, chunk pairs share one [P, 2, tw] tile
    # so A / R can be produced with one op per k-pair.
    s3t = [s3p.tile([P, 2, tw], bf16, tag="s3", name=f"s3_{th}_{blk}_{t0}_{i}")
           for i in range(4)]
    for j in range(4):
        eng = nc.gpsimd if (th == 0 and j >= 2) else nc.vector
        cj, cj4 = j, j + 4
        eng.tensor_add(s3t[cj // 2][:, cj % 2, :], s2[j][:], s2[j + 4][:])
        eng.tensor_sub(s3t[cj4 // 2][:, cj4 % 2, :], s2[j][:], s2[j + 4][:])
    # A = fp8(xh) on ACT; R = xh - A, one op per k-pair [P, 2, tw].
    # The R correction runs only on the first NKP_R k-pairs (error budget
    # verified against the gate on the exact inputs).
    for i in range(4):
        kp = blk * 4 + i
        a_ap = A[kp][:, :, ts]
        nc.scalar.activation(a_ap, s3t[i][:], ACT_F.Copy)
        if kp < NKP_R:
            eng = nc.vector if th == 0 and i % 2 == 0 else nc.gpsimd
            eng.tensor_sub(R[kp][:, :, ts], s3t[i][:], a_ap)


def _phase_m(nc, th, W1s, W2s, btile, A, R, y, wp1, wp2, yop, mps):
    for osb in range(NOS):
        _phase_m_osb(nc, th, osb, W1s, W2s, btile, A, R, y,
                     wp1, wp2, yop, mps,
                     split_evict=(th == 1 and osb == NOS - 1))


def _m_chain(nc, py, w1, w2, A, R, t0, tw, ot):
    """One PSUM group: A@W1 + A@W2 + R@W1 over tokens [t0, t0+tw)."""
    first = True
    ntc = tw // 256
    for xt, wt, isr in ((A, w1, 0), (A, w2, 0), (R, w1, 1)):
        nkp = NKP_R if isr else NKP
        for kp in range(nkp):
            for tc in range(ntc):
                out_ap = py[:, tc * 256:(tc + 1) * 256]
                tcs = slice(t0 + tc * 256, t0 + (tc + 1) * 256)
                last = (isr == 1 and kp == nkp - 1 and tc == ntc - 1)
                nc.tensor.matmul(
                    out_ap, wt[:, kp, :, ot * P:(ot + 1) * P],
                    xt[kp][:, :, tcs],
                    start=first, stop=last,
                    perf_mode=mybir.MatmulPerfMode.DoubleRow)
                first = False


def _m_load_w(nc, osb, W1s, W2s, wp1, wp2, tag):
    w1 = wp1.tile([P, NKP, 2, 512], f8, tag="w1", name=f"w1_{tag}")
    w2 = wp2.tile([P, NKP, 2, 512], f8, tag="w2", name=f"w2_{tag}")
    qk = NKP // 4
    for q in range(4):
        ksl = slice(q * qk * 1024, (q + 1) * qk * 1024)
        nc.sync.dma_start(w1[:, q * qk:(q + 1) * qk, :, :],
                          W1s.ap()[osb, :, ksl])
        nc.sync.dma_start(w2[:, q * qk:(q + 1) * qk, :, :],
                          W2s.ap()[osb, :, ksl])
    return w1, w2


def _phase_m_osb(nc, th, osb, W1s, W2s, btile, A, R, y, wp1, wp2, yop, mps,
                 split_evict=False):
    ts = slice(th * TH, (th + 1) * TH)
    w1, w2 = _m_load_w(nc, osb, W1s, W2s, wp1, wp2, f"{th}_{osb}")
    for ot in range(4):
        py = mps.tile([P, 512], f32, tag="mps", name=f"mps{th}_{osb}_{ot}")
        _m_chain(nc, py, w1, w2, A, R, th * TH, TH, ot)
        yo = yop.tile([P, TH], bf16, tag="yo", name=f"yo{th}_{osb}_{ot}")
        orow = osb * 512 + ot * P
        if split_evict:
            for hh in range(2):
                hs = slice(hh * (TH // 2), (hh + 1) * (TH // 2))
                nc.scalar.activation(
                    yo[:, hs], py[:, hs], ACT_F.Identity,
                    bias=btile[:, osb * 4 + ot:osb * 4 + ot + 1],
                    scale=1.0 / KSCALE)
                nc.sync.dma_start(
                    y.ap()[orow:orow + P,
                           th * TH + hh * (TH // 2):
                           th * TH + (hh + 1) * (TH // 2)], yo[:, hs])
        else:
            nc.scalar.activation(yo[:], py[:], ACT_F.Identity,
                                 bias=btile[:, osb * 4 + ot:osb * 4 + ot + 1],
                                 scale=1.0 / KSCALE)
            nc.sync.dma_start(y.ap()[orow:orow + P, ts], yo[:])


def build_kernel(num_devices=N_CORES, phases=('H', 'M')):
    nc = bacc.Bacc("TRN2", target_bir_lowering=False, debug=False,
                   num_devices=num_devices, dynamic_dma_scratch_size=2048)
    xT = nc.dram_tensor("xT", [NBLK, P, 8, 2, T], f8, kind="ExternalInput")
    W1s = nc.dram_tensor("W1s", [NOS, P, NKP * 2 * 512], f8,
                         kind="ExternalInput")
    W2s = nc.dram_tensor("W2s", [NOS, P, NKP * 2 * 512], f8,
                         kind="ExternalInput")
    bt = nc.dram_tensor("bt", [P, O // P], f32, kind="ExternalInput")
    Hd = nc.dram_tensor("Hd", [P, 2 * 2 * P], f8, kind="ExternalInput")
    y = nc.dram_tensor("y", [O, T], bf16, kind="ExternalOutput")

    with tile.TileContext(nc) as tc:
        with tc.tile_pool(name="const", bufs=1) as cpool, \
             tc.tile_pool(name="xA", bufs=NKP) as apool, \
             tc.tile_pool(name="xR", bufs=NKP_R) as rpool:
            hd = cpool.tile([P, 2, 2, P], f8)
            nc.sync.dma_start(hd[:], Hd.ap())
            hp = hd[:, 0, :, :]
            hn = hd[:, 1, :, :]
            btile = cpool.tile([P, O // P], f32)
            nc.sync.dma_start(btile[:], bt.ap())
            A = [apool.tile([P, 2, T], f8, tag="A", name=f"A{i}")
                 for i in range(NKP)]
            R = [rpool.tile([P, 2, T], f8, tag="R", name=f"R{i}")
                 for i in range(NKP_R)]
            with tc.tile_pool(name="xin", bufs=5) as xinp, \
                 tc.tile_pool(name="xh", bufs=18) as xhp, \
                 tc.tile_pool(name="scp", bufs=8) as scp, \
                 tc.tile_pool(name="s3p", bufs=8) as s3p, \
                 tc.tile_pool(name="wsl1", bufs=2) as wp1, \
                 tc.tile_pool(name="wsl2", bufs=2) as wp2, \
                 tc.tile_pool(name="yo", bufs=9) as yop, \
                 tc.tile_pool(name="hps", bufs=4, space="PSUM") as hps, \
                 tc.tile_pool(name="mps", bufs=4, space="PSUM") as mps:
                QW = 256
                xin0 = {}
                if 'H' in phases:
                    for q in range(2):
                        for blk in range(NBLK):
                            if q == 0:
                                xin0[blk] = _phase_h_load(nc, 0, blk, xT,
                                                          xinp)
                            _phase_h_unit(nc, 0, blk, xin0[blk], hp, hn,
                                          A, R, xhp, hps, scp, s3p,
                                          q * QW, QW)
                if 'M' in phases:
                    wq = [_m_load_w(nc, osb, W1s, W2s, wp1, wp2, f"q{osb}")
                          for osb in range(2)]
                    yoq = {}
                    for osb in range(2):
                        for ot in range(4):
                            yoq[(osb, ot)] = yop.tile(
                                [P, TH], bf16, tag="yo",
                                name=f"yoq_{osb}_{ot}")
                    for q in range(2):
                        for osb in range(2):
                            w1, w2 = wq[osb]
                            for ot in range(4):
                                py = mps.tile([P, 512], f32, tag="mps",
                                              name=f"mq_{q}_{osb}_{ot}")
                                _m_chain(nc, py, w1, w2, A, R, q * QW, QW, ot)
                                nc.scalar.activation(
                                    yoq[(osb, ot)][:, q * QW:(q + 1) * QW],
                                    py[:, 0:QW], ACT_F.Identity,
                                    bias=btile[:, osb * 4 + ot:
                                               osb * 4 + ot + 1],
                                    scale=1.0 / KSCALE)
                                if q == 1:
                                    orow = osb * 512 + ot * P
                                    nc.sync.dma_start(
                                        y.ap()[orow:orow + P, 0:TH],
                                        yoq[(osb, ot)][:])
                    for osb in range(2, NOS):
                        _phase_m_osb(nc, 0, osb, W1s, W2s, btile, A, R, y,
                                     wp1, wp2, yop, mps)
                        if 'H' in phases and 2 <= osb <= NBLK + 1:
                            blk = osb - 2
                            xin1 = _phase_h_load(nc, 1, blk, xT, xinp)
                            _phase_h_unit(nc, 1, blk, xin1, hp, hn, A, R,
                                          xhp, hps, scp, s3p, 512, TH)
                    _phase_m(nc, 1, W1s, W2s, btile, A, R, y,
                             wp1, wp2, yop, mps)
                elif 'H' in phases:
                    _phase_h(nc, 1, xT, hp, hn, A, R, xinp, xhp, hps, scp, s3p)
    nc.compile()
    return nc


_CACHED_NC = None


def _get_nc():
    global _CACHED_NC
    if _CACHED_NC is None:
        _CACHED_NC = build_kernel()
    return _CACHED_NC


def _hadamard128():
    h = np.array([[1.0]], dtype=np.float32)
    while h.shape[0] < P:
        h = np.block([[h, h], [h, -h]])
    return h.astype(np.float32)


def _w_slabs(Wq):
    Wt = Wq.T.reshape(NKP, 2, P, O).transpose(2, 0, 1, 3)
    Wt = Wt.reshape(P, NKP, 2, NOS, 512).transpose(3, 0, 1, 2, 4)
    return np.ascontiguousarray(Wt.reshape(NOS, P, NKP * 2 * 512))


def kernel(x, W, b):
    x = np.asarray(x, dtype=np.float32)
    W = np.asarray(W, dtype=np.float32)
    b = np.asarray(b, dtype=np.float32)
    assert x.shape == (B, S, D) and W.shape == (O, D) and b.shape == (O,)

    nc = _get_nc()
    Ws = (W * KSCALE).astype(np.float32)
    W1 = Ws.astype(E4)
    W2 = (Ws - W1.astype(np.float32)).astype(E4)
    W1s, W2s = _w_slabs(W1), _w_slabs(W2)
    btH = np.ascontiguousarray(b.reshape(O // P, P).T).astype(np.float32)
    h = _hadamard128() / 32.0
    HdH = np.stack([np.stack([h, h], axis=1),
                    np.stack([h, -h], axis=1)], axis=1)  # [P, sign, slot, P]
    HdH = np.ascontiguousarray(HdH.reshape(P, 2 * 2 * P)).astype(E4)

    xf = x.reshape(B * S, D)
    in_maps = []
    for c in range(N_CORES):
        xs = xf[c * T:(c + 1) * T]
        xTl = xs.T.reshape(NBLK, 8, P, T).transpose(0, 2, 1, 3)  # [blk,p,c,t]
        x1 = xTl.astype(E4)
        x2 = (xTl - x1.astype(np.float32)).astype(E4)
        xpair = np.ascontiguousarray(
            np.stack([x1, x2], axis=3))              # [blk, p, c, 2, t] fp8
        in_maps.append({"xT": xpair, "W1s": W1s, "W2s": W2s,
                        "bt": btH, "Hd": HdH})
    res = run_bass_kernel_spmd(nc, in_maps, core_ids=list(range(N_CORES)))
    yT = np.concatenate(
        [np.asarray(res.results[c]["y"]).astype(np.float32)
         for c in range(N_CORES)], axis=1)
    return np.ascontiguousarray(yT.T).reshape(B, S, O)
